# revision 18
# baseline (speedup 1.0000x reference)
"""Multi-head attention (B=4, L=2048, E=1024, H=16, causal) for 8 Trainium2
NeuronCores.

Sharding: data-parallel over batch (4) x tensor-parallel over heads (2 groups
of 8 heads).  Core c handles batch c//2, head-group c%2.  Each core runs the
q/k/v projections for its 8 heads (column shards of wq/wk/wv), causal
flash-style attention, and its row-shard of the output projection; the
all-reduce over the two head-groups is the final gather (host-side add).

On-chip layout (per core):
  QT/KT  [128p, 4, 2048]  fp16  Q^T / K^T: partition = head-pair dq (2x64), free = seq
  V      [128p, 16, 8, 65] fp16 V: partition = seq chunk, per-head 64 dv + ones col
  scores S^T[k, q] = K @ Q^T computed per 128-wide k chunk as PE matmuls
  (contract dim 64, head pairs packed in partition halves), exp on ACT with
  fused 1/sqrt(dk) scale, denominator via the V ones-column, causal handled by
  skipping q < k blocks + 4 precomputed diagonal masks.
"""

import numpy as np

import concourse.bass as bass
import concourse.mybir as mybir
import concourse.tile as tile
from concourse import bacc
from concourse.bass_utils import run_bass_kernel_spmd

# ---------------------------------------------------------------------------
# Problem constants (hardcoded per the harness contract)
# ---------------------------------------------------------------------------
B, L, E, H = 4, 2048, 1024, 16
DK = E // H          # 64
NCORES = 8
HL = H // 2          # heads per core = 8
DQ = HL * DK         # 512 = per-core projection width
P = 128
EC = E // P          # 8 contraction chunks
NLT = L // 512       # 4 l-tiles of 512
NKC = L // P         # 16 k chunks of 128
NDC = DQ // P        # 4 dq chunks (head pairs)
F16 = mybir.dt.float16
F32 = mybir.dt.float32

_BUILT = {}


def _build(causal: bool):
    nc = bacc.Bacc("TRN2", num_devices=NCORES, debug=False)

    qT = nc.dram_tensor("qT", [P, EC, L], F16, kind="ExternalInput")
    kT = nc.dram_tensor("kT", [P, EC, L], F16, kind="ExternalInput")
    vT = nc.dram_tensor("vT", [P, EC, L], F16, kind="ExternalInput")
    wqT = nc.dram_tensor("wqT", [P, EC, DQ], F16, kind="ExternalInput")
    wkT = nc.dram_tensor("wkT", [P, EC, DQ], F16, kind="ExternalInput")
    wvT = nc.dram_tensor("wvT", [P, EC, DQ], F16, kind="ExternalInput")
    woT = nc.dram_tensor("woT", [P, NDC, E], F16, kind="ExternalInput")
    bq = nc.dram_tensor("bq", [P, NDC], F32, kind="ExternalInput")
    bk = nc.dram_tensor("bk", [P, NDC], F32, kind="ExternalInput")
    bv = nc.dram_tensor("bv", [1, DQ], F16, kind="ExternalInput")
    bo = nc.dram_tensor("bo", [1, E], F16, kind="ExternalInput")
    masks = nc.dram_tensor("masks", [P, P], F16, kind="ExternalInput")
    out = nc.dram_tensor("out", [L, E], F32, kind="ExternalOutput")

    with tile.TileContext(nc) as tc:
        with (
            tc.tile_pool(name="const", bufs=1) as const,
            tc.tile_pool(name="persist", bufs=1) as persist,
            tc.tile_pool(name="stage", bufs=3) as stage,
            tc.tile_pool(name="pt", bufs=6) as ptp,
            tc.tile_pool(name="small", bufs=3) as small,
            tc.tile_pool(name="osb", bufs=3) as osb,
            tc.tile_pool(name="psum_big", bufs=2, space="PSUM") as psb,
            tc.tile_pool(name="psum_ctx", bufs=4, space="PSUM") as psc,
            tc.tile_pool(name="dscratch", bufs=4, space="DRAM") as dsp,
        ):
            # ---- constants -------------------------------------------------
            wv_sb = const.tile([P, EC, DQ], F16, tag="wv")
            wk_sb = const.tile([P, EC, DQ], F16, tag="wk")
            wq_sb = const.tile([P, EC, DQ], F16, tag="wq")
            wo_sb = const.tile([P, NDC, E], F16, tag="wo")
            bq_sb = const.tile([P, NDC], F32, tag="bq")
            bk_sb = const.tile([P, NDC], F32, tag="bk")
            bv_sb = const.tile([1, DQ], F16, tag="bv")
            bo_sb = const.tile([1, E], F16, tag="bo")
            mask_sb = const.tile([P, P], F16, tag="masks")
            ones_sb = const.tile([1, P], F16, tag="ones")
            nc.sync.dma_start(bv_sb[:], bv[:])
            nc.sync.dma_start(bk_sb[:], bk[:])
            nc.sync.dma_start(bq_sb[:], bq[:])
            nc.sync.dma_start(mask_sb[:], masks[:])
            nc.sync.dma_start(wv_sb[:], wvT[:])
            nc.scalar.dma_start(wk_sb[:], wkT[:])
            nc.gpsimd.dma_start(wq_sb[:], wqT[:])
            nc.vector.memset(ones_sb[:], 1.0)

            # ---- persistent activations -----------------------------------
            QT_sb = persist.tile([P, NDC, L], F16, tag="QT")
            KT_sb = persist.tile([P, NDC, L], F16, tag="KT")
            CT_sb = persist.tile([P, NDC, L], F16, tag="CT")   # ctx^T, normalized
            V_sb = persist.tile([P, NKC, HL, DK + 1], F16, tag="V")
            nc.vector.memset(V_sb[:, :, :, DK], 1.0)

            # q^T input stays resident so Q projections can interleave with
            # attention (one dq chunk just-in-time per head pair); loaded
            # after the latency-critical V/K-projection DMAs are queued.
            qTf = [
                persist.tile([P, EC, 512], F16, tag=f"qTf{lt}", name=f"qTf_{lt}")
                for lt in range(NLT)
            ]

            # ---- projection group emitters --------------------------------
            def emit_vproj(lc):
                vt = stage.tile([P, EC, P], F16, tag="vstage")
                nc.sync.dma_start(vt[:], vT[:, :, lc * P : (lc + 1) * P])
                ps = psb.tile([P, 1024], F32, tag="big", name=f"vps_{lc}")
                for ec in range(EC):
                    nc.tensor.matmul(
                        ps[:, :512],
                        vt[:, ec, :],
                        wv_sb[:, ec, :],
                        start=(ec == 0),
                        stop=False,
                    )
                nc.tensor.matmul(
                    ps[:, :512], ones_sb[0:1, :], bv_sb[0:1, :], start=False, stop=True
                )
                nc.vector.tensor_copy(
                    V_sb[:, lc, :, 0:DK],
                    ps[:, :512].rearrange("p (h d) -> p h d", h=HL),
                )

            def emit_qproj(dc, lt):
                ps = psb.tile([P, 1024], F32, tag="big", name=f"qps_{dc}_{lt}")
                for ec in range(EC):
                    nc.tensor.matmul(
                        ps[:, :512],
                        wq_sb[:, ec, dc * P : (dc + 1) * P],
                        qTf[lt][:, ec, :],
                        start=(ec == 0),
                        stop=(ec == EC - 1),
                    )
                nc.vector.tensor_scalar_add(
                    QT_sb[:, dc, lt * 512 : (lt + 1) * 512],
                    ps[:, :512],
                    bq_sb[:, dc : dc + 1],
                )

            for lt in range(NLT):
                nc.gpsimd.dma_start(qTf[lt][:], qT[:, :, lt * 512 : (lt + 1) * 512])
            nc.scalar.dma_start(wo_sb[:], woT[:])
            nc.sync.dma_start(bo_sb[:], bo[:])

            # V for the first two k chunks, then K (streamed), then Q chunk 0
            emit_vproj(0)
            emit_vproj(1)
            # ---- K projection (all chunks, streamed input) ----------------
            for lt in range(NLT):
                xt = stage.tile([P, EC, 512], F16, tag="xstage")
                nc.scalar.dma_start(xt[:], kT[:, :, lt * 512 : (lt + 1) * 512])
                for dc in range(NDC):
                    ps = psb.tile([P, 1024], F32, tag="big", name=f"kps_{dc}_{lt}")
                    for ec in range(EC):
                        nc.tensor.matmul(
                            ps[:, :512],
                            wk_sb[:, ec, dc * P : (dc + 1) * P],
                            xt[:, ec, :],
                            start=(ec == 0),
                            stop=(ec == EC - 1),
                        )
                    nc.vector.tensor_scalar_add(
                        KT_sb[:, dc, lt * 512 : (lt + 1) * 512],
                        ps[:, :512],
                        bk_sb[:, dc : dc + 1],
                    )

            for lt in range(NLT):
                emit_qproj(0, lt)

            # PE filler work woven into the (ACT-bound) attention pair loops:
            # pair 0 finishes the V projection and projects Q chunk 1; pairs
            # 1/2 project Q chunks 2/3.
            fillers = {
                0: [lambda lc=lc: emit_vproj(lc) for lc in range(2, NKC)]
                + [lambda lt=lt: emit_qproj(1, lt) for lt in range(NLT)],
                1: [lambda lt=lt: emit_qproj(2, lt) for lt in range(NLT)],
                2: [lambda lt=lt: emit_qproj(3, lt) for lt in range(NLT)],
                3: [],
            }

            # ---- attention, head pairs x 1024-wide q groups ----------------
            # Head pair (2hp, 2hp+1) lives in partition halves 0/64 of chunk
            # hp, so their S^T matmuls (contract dim 64) go to disjoint PE row
            # groups and run concurrently.  Stripes start at the causal
            # diagonal (q0 = kj*128); the triangular [128,128] mask covers the
            # first 128 columns.  PV for stripe kj-1 is emitted after the
            # ST+exp of stripe kj so the PE never sits behind a not-yet-exped
            # stripe in its queue.
            scale = float(1.0 / np.sqrt(DK))
            for hp in range(NDC):
                heads = (2 * hp, 2 * hp + 1)
                filler = fillers[hp]
                for grp in range(2):
                    glo, ghi = grp * 1024, grp * 1024 + 1024
                    qis = (2 * grp, 2 * grp + 1)
                    n_kj = (8 * grp + 8) if causal else NKC
                    ctx_ps = {
                        (h, qi): psc.tile(
                            [DK + 1, 512], F32, tag="ctx", name=f"ctx_{h}_{grp}_{qi}"
                        )
                        for h in heads
                        for qi in qis
                    }

                    def emit_divide(h, qi):
                        # normalize ctx^T by the denominator row (psum
                        # partition 64) and store as fp16.  The denominator is
                        # broadcast to partitions 0..63 by bouncing through a
                        # DRAM scratch row (DMA can broadcast from DRAM but
                        # not from SBUF/PSUM).
                        g0 = 64 * (h % 2)
                        cps = ctx_ps[(h, qi)]
                        drow = small.tile([P, 512], F32, tag="rec")
                        nc.vector.tensor_copy(
                            drow[DK : DK + 1, :], cps[DK : DK + 1, :]
                        )
                        dsc = dsp.tile([1, 512], F32, name=f"dsc_{h}_{qi}", tag="dsc")
                        nc.sync.dma_start(dsc[:], drow[DK : DK + 1, :])
                        den64 = small.tile([64, 512], F32, tag="recb")
                        nc.sync.dma_start(
                            den64[:], dsc[0:1, :].to_broadcast((64, 512))
                        )
                        rec64 = small.tile([64, 512], F32, tag="rec64")
                        nc.vector.reciprocal_approx_fast(rec64[:], den64[:])
                        qs = slice(qi * 512, (qi + 1) * 512)
                        if g0 == 0:
                            nc.vector.tensor_mul(
                                CT_sb[0:64, hp, qs], cps[0:DK, :], rec64[:]
                            )
                        else:
                            tmp = small.tile([64, 512], F16, tag="ctmp")
                            nc.vector.tensor_mul(tmp[:], cps[0:DK, :], rec64[:])
                            nc.sync.dma_start(CT_sb[64:128, hp, qs], tmp[:])

                    def emit_st(kj):
                        pts = {}
                        for h in heads:
                            g0 = 64 * (h % 2)
                            q0 = max(glo, kj * P) if causal else glo
                            W = ghi - q0
                            # matmul outputs must stay within one PSUM bank:
                            # anchor the stripe so tile col = q - base with
                            # base 512-aligned.
                            base = (q0 // 512) * 512
                            st = psb.tile(
                                [P, 1024], F32, tag="big", name=f"st_{h}_{grp}_{kj}"
                            )
                            j = q0
                            while j < ghi:
                                w = min(512 - (j % 512), ghi - j)
                                nc.tensor.matmul(
                                    st[:, j - base : j - base + w],
                                    KT_sb[g0 : g0 + 64, hp, kj * P : (kj + 1) * P],
                                    QT_sb[g0 : g0 + 64, hp, j : j + w],
                                    start=True,
                                    stop=True,
                                )
                                j += w
                            pt = ptp.tile([P, 1024], F16, tag="pt", name=f"pt_{h}_{kj}")
                            nc.scalar.activation(
                                pt[:, q0 - base : q0 - base + W],
                                st[:, q0 - base : q0 - base + W],
                                mybir.ActivationFunctionType.Exp,
                                scale=scale,
                            )
                            if causal and kj * P >= glo:
                                nc.vector.tensor_mul(
                                    pt[:, q0 - base : q0 - base + P],
                                    pt[:, q0 - base : q0 - base + P],
                                    mask_sb[:, 0:P],
                                )
                            pts[h] = (pt, q0, base)
                        return pts

                    def emit_pv(kj, pts):
                        for h in heads:
                            pt, q0, base = pts[h]
                            for qi in qis:
                                lo = max(qi * 512, q0)
                                w = qi * 512 + 512 - lo
                                if w <= 0:
                                    continue
                                last = (4 * qi + 3) if causal else (NKC - 1)
                                if kj > last:
                                    continue
                                nc.tensor.matmul(
                                    ctx_ps[(h, qi)][:, lo - qi * 512 :],
                                    V_sb[:, kj, h, :],
                                    pt[:, lo - base : lo - base + w],
                                    start=(kj == 0),
                                    stop=(kj == last),
                                )
                                if kj == last:
                                    emit_divide(h, qi)

                    prev = None
                    for kj in range(n_kj):
                        pts = emit_st(kj)
                        if prev is not None:
                            emit_pv(prev[0], prev[1])
                        # pair 0's fillers include V projections that PV
                        # matmuls of later kj read — keep them a step ahead
                        # (pop every iteration); Q-only fillers go slower.
                        if filler and (hp == 0 or kj % 2 == 0):
                            filler.pop(0)()
                        prev = (kj, pts)
                    emit_pv(prev[0], prev[1])
                # any leftover filler after both groups of this pair
                while filler:
                    filler.pop(0)()

            # ---- output projection: out[q, e] = ctx @ wo_shard^T + bo -----
            for qc in range(L // P):
                for eh in range(E // 512):
                    ps = psb.tile([P, 1024], F32, tag="big", name=f"ops_{qc}_{eh}")
                    for dc in range(NDC):
                        nc.tensor.matmul(
                            ps[:, :512],
                            CT_sb[:, dc, qc * P : (qc + 1) * P],
                            wo_sb[:, dc, eh * 512 : (eh + 1) * 512],
                            start=(dc == 0),
                            stop=False,
                        )
                    nc.tensor.matmul(
                        ps[:, :512],
                        ones_sb[0:1, :],
                        bo_sb[0:1, eh * 512 : (eh + 1) * 512],
                        start=False,
                        stop=True,
                    )
                    ot = osb.tile([P, 512], F32, tag="ot")
                    nc.vector.tensor_copy(ot[:], ps[:, :512])
                    nc.sync.dma_start(
                        out[qc * P : (qc + 1) * P, eh * 512 : (eh + 1) * 512], ot[:]
                    )

    nc.compile()
    return nc


def _chunked(x, inner):
    """[outer*inner, n] -> [inner, outer, n] with element [p, c, n] = x[c*inner+p, n]."""
    o = x.shape[0] // inner
    return np.ascontiguousarray(x.reshape(o, inner, *x.shape[1:]).transpose(1, 0, 2))


def kernel(query, key, value, wq, bq, wk, bk, wv, bv, wo, bo, is_causal):
    return _run(query, key, value, wq, bq, wk, bk, wv, bv, wo, bo, is_causal)[0]


def _run(query, key, value, wq, bq, wk, bk, wv, bv, wo, bo, is_causal, **run_kwargs):
    query = np.asarray(query, dtype=np.float32)
    key = np.asarray(key, dtype=np.float32)
    value = np.asarray(value, dtype=np.float32)
    wq, wk, wv, wo = (np.asarray(w, dtype=np.float32) for w in (wq, wk, wv, wo))
    bq, bk, bv, bo = (np.asarray(b, dtype=np.float32) for b in (bq, bk, bv, bo))
    causal = bool(int(np.asarray(is_causal)))

    if causal not in _BUILT:
        _BUILT[causal] = _build(causal)
    nc = _BUILT[causal]

    # triangular diagonal mask: mask[kp, qf] = 1 if qf >= kp
    qf = np.arange(P)[None, :]
    kp = np.arange(P)[:, None]
    masks_np = (qf >= kp).astype(np.float16)                  # [128, 128]

    in_maps = []
    for c in range(NCORES):
        b = c // 2
        g = c % 2
        hs = slice(g * DQ, (g + 1) * DQ)
        m = {
            "qT": _chunked(query[b].T.astype(np.float16), P),
            "kT": _chunked(key[b].T.astype(np.float16), P),
            "vT": _chunked(value[b].T.astype(np.float16), P),
            "wqT": _chunked(wq[hs, :].T.astype(np.float16), P),
            "wkT": _chunked(wk[hs, :].T.astype(np.float16), P),
            "wvT": _chunked(wv[hs, :].T.astype(np.float16), P),
            "woT": _chunked(wo[:, hs].T.astype(np.float16), P),
            "bq": np.ascontiguousarray(bq[hs].reshape(NDC, P).T.astype(np.float32)),
            "bk": np.ascontiguousarray(bk[hs].reshape(NDC, P).T.astype(np.float32)),
            "bv": bv[hs].reshape(1, DQ).astype(np.float16),
            "bo": (bo if g == 0 else np.zeros_like(bo)).reshape(1, E).astype(np.float16),
            "masks": masks_np,
        }
        in_maps.append(m)

    res = run_bass_kernel_spmd(nc, in_maps, core_ids=list(range(NCORES)), **run_kwargs)

    out = np.empty((B, L, E), dtype=np.float32)
    for b in range(B):
        out[b] = res.results[2 * b]["out"] + res.results[2 * b + 1]["out"]
    return out, res


# revision 27
# speedup vs baseline: 1.0293x; 1.0293x over previous
"""Multi-head attention (B=4, L=2048, E=1024, H=16, causal) for 8 Trainium2
NeuronCores.

Sharding: data-parallel over batch (4) x tensor-parallel over heads (2 groups
of 8 heads).  Core c handles batch c//2, head-group c%2.  Each core runs the
q/k/v projections for its 8 heads (column shards of wq/wk/wv), causal
flash-style attention, and its row-shard of the output projection; the
all-reduce over the two head-groups is the final gather (host-side add).

On-chip layout (per core):
  QT/KT  [128p, 4, 2048]  fp16  Q^T / K^T: partition = head-pair dq (2x64), free = seq
  V      [128p, 16, 8, 65] fp16 V: partition = seq chunk, per-head 64 dv + ones col
  scores S^T[k, q] = K @ Q^T computed per 128-wide k chunk as PE matmuls
  (contract dim 64, head pairs packed in partition halves), exp on ACT with
  fused 1/sqrt(dk) scale, denominator via the V ones-column, causal handled by
  skipping q < k blocks + 4 precomputed diagonal masks.
"""

import numpy as np

import concourse.bass as bass
import concourse.mybir as mybir
import concourse.tile as tile
from concourse import bacc
from concourse.bass_utils import run_bass_kernel_spmd

# ---------------------------------------------------------------------------
# Problem constants (hardcoded per the harness contract)
# ---------------------------------------------------------------------------
B, L, E, H = 4, 2048, 1024, 16
DK = E // H          # 64
NCORES = 8
HL = H // 2          # heads per core = 8
DQ = HL * DK         # 512 = per-core projection width
P = 128
EC = E // P          # 8 contraction chunks
NLT = L // 512       # 4 l-tiles of 512
NKC = L // P         # 16 k chunks of 128
NDC = DQ // P        # 4 dq chunks (head pairs)
F16 = mybir.dt.float16
F32 = mybir.dt.float32

_BUILT = {}


def _build(causal: bool):
    nc = bacc.Bacc("TRN2", num_devices=NCORES, debug=False)

    qT = nc.dram_tensor("qT", [P, EC, L], F16, kind="ExternalInput")
    kT = nc.dram_tensor("kT", [P, EC, L], F16, kind="ExternalInput")
    vT = nc.dram_tensor("vT", [P, EC, L], F16, kind="ExternalInput")
    wqT = nc.dram_tensor("wqT", [P, EC, DQ], F16, kind="ExternalInput")
    wkT = nc.dram_tensor("wkT", [P, EC, DQ], F16, kind="ExternalInput")
    wvT = nc.dram_tensor("wvT", [P, EC, DQ], F16, kind="ExternalInput")
    woT = nc.dram_tensor("woT", [P, NDC, E], F16, kind="ExternalInput")
    bq = nc.dram_tensor("bq", [P, NDC], F32, kind="ExternalInput")
    bk = nc.dram_tensor("bk", [P, NDC], F32, kind="ExternalInput")
    bv = nc.dram_tensor("bv", [1, DQ], F16, kind="ExternalInput")
    bo = nc.dram_tensor("bo", [1, E], F16, kind="ExternalInput")
    masks = nc.dram_tensor("masks", [P, P], F16, kind="ExternalInput")
    out = nc.dram_tensor("out", [L, E], F32, kind="ExternalOutput")

    with tile.TileContext(nc) as tc:
        with (
            tc.tile_pool(name="const", bufs=1) as const,
            tc.tile_pool(name="persist", bufs=1) as persist,
            tc.tile_pool(name="stage", bufs=3) as stage,
            tc.tile_pool(name="pt", bufs=6) as ptp,
            tc.tile_pool(name="small", bufs=3) as small,
            tc.tile_pool(name="osb", bufs=3) as osb,
            tc.tile_pool(name="psum_big", bufs=2, space="PSUM") as psb,
            tc.tile_pool(name="psum_ctx", bufs=4, space="PSUM") as psc,
            tc.tile_pool(name="dscratch", bufs=4, space="DRAM") as dsp,
        ):
            # ---- constants -------------------------------------------------
            wv_lo = const.tile([P, EC // 2, DQ], F16, tag="wvlo")
            wv_hi = const.tile([P, EC // 2, DQ], F16, tag="wvhi")
            wk_lo = const.tile([P, EC // 2, DQ], F16, tag="wklo")
            wk_hi = const.tile([P, EC // 2, DQ], F16, tag="wkhi")
            wq_sb = const.tile([P, EC, DQ], F16, tag="wq")
            wo_sb = const.tile([P, NDC, E], F16, tag="wo")
            bq_sb = const.tile([P, NDC], F32, tag="bq")
            bk_sb = const.tile([P, NDC], F32, tag="bk")
            mask_sb = const.tile([P, P], F16, tag="masks")
            ones64_sb = const.tile([P, 64], F16, tag="ones64")
            bvb_sb = const.tile([P, DQ], F16, tag="bvb")
            bob_sb = const.tile([P, E], F16, tag="bob")
            nc.sync.dma_start(bk_sb[:], bk[:])
            nc.sync.dma_start(bq_sb[:], bq[:])
            nc.sync.dma_start(mask_sb[:], masks[:])
            nc.sync.dma_start(wv_lo[:], wvT[:, 0 : EC // 2, :])
            nc.scalar.dma_start(wv_hi[:], wvT[:, EC // 2 :, :])
            nc.sync.dma_start(wk_lo[:], wkT[:, 0 : EC // 2, :])
            nc.scalar.dma_start(wk_hi[:], wkT[:, EC // 2 :, :])
            nc.gpsimd.dma_start(wq_sb[:], wqT[:])
            nc.vector.memset(ones64_sb[:], 1.0)
            # bias broadcast tiles (bias add via DVE instead of extra matmuls)
            nc.sync.dma_start(bvb_sb[:], bv[0:1, :].to_broadcast((P, DQ)))
            nc.sync.dma_start(bob_sb[:], bo[0:1, :].to_broadcast((P, E)))

            # ---- persistent activations -----------------------------------
            QT_sb = persist.tile([P, NDC, L], F16, tag="QT")
            KT_sb = persist.tile([P, NDC, L], F16, tag="KT")
            CT_sb = persist.tile([P, NDC, L], F16, tag="CT")   # ctx^T, normalized
            V_sb = persist.tile([P, NKC, HL, DK + 1], F16, tag="V")
            nc.vector.memset(V_sb[:, :, :, DK], 1.0)

            # q^T input stays resident so Q projections can interleave with
            # attention (one dq chunk just-in-time per head pair); loaded
            # after the latency-critical V/K-projection DMAs are queued.
            qTf = [
                persist.tile([P, EC, 512], F16, tag=f"qTf{lt}", name=f"qTf_{lt}")
                for lt in range(NLT)
            ]

            # ---- projection group emitters --------------------------------
            def emit_vproj(lc):
                vt = stage.tile([P, EC, P], F16, tag="vstage")
                nc.sync.dma_start(vt[:], vT[:, :, lc * P : (lc + 1) * P])
                ps = psb.tile([P, 1024], F32, tag="big", name=f"vps_{lc}")
                for ec in range(EC):
                    w = wv_lo if ec < EC // 2 else wv_hi
                    nc.tensor.matmul(
                        ps[:, :512],
                        vt[:, ec, :],
                        w[:, ec % (EC // 2), :],
                        start=(ec == 0),
                        stop=(ec == EC - 1),
                    )
                nc.vector.tensor_add(
                    V_sb[:, lc, :, 0:DK],
                    ps[:, :512].rearrange("p (h d) -> p h d", h=HL),
                    bvb_sb[:].rearrange("p (h d) -> p h d", h=HL),
                )

            def emit_qproj(dc, lt):
                ps = psb.tile([P, 1024], F32, tag="big", name=f"qps_{dc}_{lt}")
                for ec in range(EC):
                    nc.tensor.matmul(
                        ps[:, :512],
                        wq_sb[:, ec, dc * P : (dc + 1) * P],
                        qTf[lt][:, ec, :],
                        start=(ec == 0),
                        stop=(ec == EC - 1),
                    )
                nc.vector.tensor_scalar_add(
                    QT_sb[:, dc, lt * 512 : (lt + 1) * 512],
                    ps[:, :512],
                    bq_sb[:, dc : dc + 1],
                )

            for lt in range(NLT):
                nc.gpsimd.dma_start(qTf[lt][:], qT[:, :, lt * 512 : (lt + 1) * 512])
            nc.scalar.dma_start(wo_sb[:], woT[:])

            # V for the first two k chunks, then K (streamed), then Q chunk 0
            emit_vproj(0)
            emit_vproj(1)
            # ---- K projection (all chunks, streamed input) ----------------
            for lt in range(NLT):
                xt = stage.tile([P, EC, 512], F16, tag="xstage")
                nc.scalar.dma_start(xt[:], kT[:, :, lt * 512 : (lt + 1) * 512])
                for dc in range(NDC):
                    ps = psb.tile([P, 1024], F32, tag="big", name=f"kps_{dc}_{lt}")
                    for ec in range(EC):
                        w = wk_lo if ec < EC // 2 else wk_hi
                        nc.tensor.matmul(
                            ps[:, :512],
                            w[:, ec % (EC // 2), dc * P : (dc + 1) * P],
                            xt[:, ec, :],
                            start=(ec == 0),
                            stop=(ec == EC - 1),
                        )
                    nc.vector.tensor_scalar_add(
                        KT_sb[:, dc, lt * 512 : (lt + 1) * 512],
                        ps[:, :512],
                        bk_sb[:, dc : dc + 1],
                    )

            for lt in range(NLT):
                emit_qproj(0, lt)

            # PE filler work woven into the (ACT-bound) attention pair loops:
            # pair 0 finishes the V projection and projects Q chunk 1; pairs
            # 1/2 project Q chunks 2/3.
            fillers = {
                0: [lambda lc=lc: emit_vproj(lc) for lc in range(2, NKC)]
                + [lambda lt=lt: emit_qproj(1, lt) for lt in range(NLT)],
                1: [lambda lt=lt: emit_qproj(2, lt) for lt in range(NLT)],
                2: [lambda lt=lt: emit_qproj(3, lt) for lt in range(NLT)],
                3: [],
            }

            # ---- attention, head pairs x 1024-wide q groups ----------------
            # Head pair (2hp, 2hp+1) lives in partition halves 0/64 of chunk
            # hp, so their S^T matmuls (contract dim 64) go to disjoint PE row
            # groups and run concurrently.  Stripes start at the causal
            # diagonal (q0 = kj*128); the triangular [128,128] mask covers the
            # first 128 columns.  PV for stripe kj-1 is emitted after the
            # ST+exp of stripe kj so the PE never sits behind a not-yet-exped
            # stripe in its queue.
            scale = float(1.0 / np.sqrt(DK))
            for hp in range(NDC):
                heads = (2 * hp, 2 * hp + 1)
                filler = fillers[hp]
                for grp in range(2):
                    glo, ghi = grp * 1024, grp * 1024 + 1024
                    qis = (2 * grp, 2 * grp + 1)
                    n_kj = (8 * grp + 8) if causal else NKC
                    ctx_ps = {
                        (h, qi): psc.tile(
                            [DK + 1, 512], F32, tag="ctx", name=f"ctx_{h}_{grp}_{qi}"
                        )
                        for h in heads
                        for qi in qis
                    }

                    def emit_divide(h, qi):
                        # normalize ctx^T by the denominator row (psum
                        # partition 64) and store as fp16.  The denominator is
                        # broadcast to partitions 0..63 by bouncing through a
                        # DRAM scratch row (DMA can broadcast from DRAM but
                        # not from SBUF/PSUM).
                        g0 = 64 * (h % 2)
                        cps = ctx_ps[(h, qi)]
                        # 1/denom on the denominator row (psum partition 64),
                        # then replicate to partitions 0..63 with a rank-1
                        # matmul (ones[1,64] at base partition 64).
                        drow = small.tile([P, 512], F32, tag="rec")
                        nc.vector.tensor_copy(
                            drow[DK : DK + 1, :], cps[DK : DK + 1, :]
                        )
                        dsc = dsp.tile([1, 512], F32, name=f"dsc_{h}_{qi}", tag="dsc")
                        nc.sync.dma_start(dsc[:], drow[DK : DK + 1, :])
                        den64 = small.tile([64, 512], F32, tag="recb")
                        nc.sync.dma_start(
                            den64[:], dsc[0:1, :].to_broadcast((64, 512))
                        )
                        rec64 = small.tile([64, 512], F32, tag="rec64")
                        nc.vector.reciprocal_approx_fast(rec64[:], den64[:])
                        qs = slice(qi * 512, (qi + 1) * 512)
                        if g0 == 0:
                            nc.vector.tensor_mul(
                                CT_sb[0:64, hp, qs], cps[0:DK, :], rec64[:]
                            )
                        else:
                            tmp = small.tile([64, 512], F16, tag="ctmp")
                            nc.vector.tensor_mul(tmp[:], cps[0:DK, :], rec64[:])
                            nc.sync.dma_start(CT_sb[64:128, hp, qs], tmp[:])

                    def emit_st(kj):
                        pts = {}
                        for h in heads:
                            g0 = 64 * (h % 2)
                            q0 = max(glo, kj * P) if causal else glo
                            W = ghi - q0
                            # matmul outputs must stay within one PSUM bank:
                            # anchor the stripe so tile col = q - base with
                            # base 512-aligned.
                            base = (q0 // 512) * 512
                            st = psb.tile(
                                [P, 1024], F32, tag="big", name=f"st_{h}_{grp}_{kj}"
                            )
                            j = q0
                            while j < ghi:
                                w = min(512 - (j % 512), ghi - j)
                                nc.tensor.matmul(
                                    st[:, j - base : j - base + w],
                                    KT_sb[g0 : g0 + 64, hp, kj * P : (kj + 1) * P],
                                    QT_sb[g0 : g0 + 64, hp, j : j + w],
                                    start=True,
                                    stop=True,
                                )
                                j += w
                            pt = ptp.tile([P, 1024], F16, tag="pt", name=f"pt_{h}_{kj}")
                            nc.scalar.activation(
                                pt[:, q0 - base : q0 - base + W],
                                st[:, q0 - base : q0 - base + W],
                                mybir.ActivationFunctionType.Exp,
                                scale=scale,
                            )
                            if causal and kj * P >= glo:
                                nc.vector.tensor_mul(
                                    pt[:, q0 - base : q0 - base + P],
                                    pt[:, q0 - base : q0 - base + P],
                                    mask_sb[:, 0:P],
                                )
                            pts[h] = (pt, q0, base)
                        return pts

                    def emit_pv(kj, pts):
                        for h in heads:
                            pt, q0, base = pts[h]
                            for qi in qis:
                                lo = max(qi * 512, q0)
                                w = qi * 512 + 512 - lo
                                if w <= 0:
                                    continue
                                last = (4 * qi + 3) if causal else (NKC - 1)
                                if kj > last:
                                    continue
                                nc.tensor.matmul(
                                    ctx_ps[(h, qi)][:, lo - qi * 512 :],
                                    V_sb[:, kj, h, :],
                                    pt[:, lo - base : lo - base + w],
                                    start=(kj == 0),
                                    stop=(kj == last),
                                )
                                if kj == last:
                                    emit_divide(h, qi)

                    prev = None
                    for kj in range(n_kj):
                        pts = emit_st(kj)
                        if prev is not None:
                            emit_pv(prev[0], prev[1])
                        # pair 0's fillers include V projections that PV
                        # matmuls of later kj read — keep them a step ahead
                        # (pop every iteration); Q-only fillers go slower.
                        if filler and (hp == 0 or kj % 2 == 0):
                            filler.pop(0)()
                        prev = (kj, pts)
                    emit_pv(prev[0], prev[1])
                # any leftover filler after both groups of this pair
                while filler:
                    filler.pop(0)()

            # ---- output projection: out[q, e] = ctx @ wo_shard^T + bo -----
            for qc in range(L // P):
                for eh in range(E // 512):
                    ps = psb.tile([P, 1024], F32, tag="big", name=f"ops_{qc}_{eh}")
                    for dc in range(NDC):
                        nc.tensor.matmul(
                            ps[:, :512],
                            CT_sb[:, dc, qc * P : (qc + 1) * P],
                            wo_sb[:, dc, eh * 512 : (eh + 1) * 512],
                            start=(dc == 0),
                            stop=(dc == NDC - 1),
                        )
                    ot = osb.tile([P, 512], F32, tag="ot")
                    nc.vector.tensor_add(
                        ot[:], ps[:, :512], bob_sb[:, eh * 512 : (eh + 1) * 512]
                    )
                    eng = nc.sync if (qc + eh) % 2 == 0 else nc.scalar
                    eng.dma_start(
                        out[qc * P : (qc + 1) * P, eh * 512 : (eh + 1) * 512], ot[:]
                    )

    nc.compile()
    return nc


def _chunked(x, inner):
    """[outer*inner, n] -> [inner, outer, n] with element [p, c, n] = x[c*inner+p, n]."""
    o = x.shape[0] // inner
    return np.ascontiguousarray(x.reshape(o, inner, *x.shape[1:]).transpose(1, 0, 2))


def kernel(query, key, value, wq, bq, wk, bk, wv, bv, wo, bo, is_causal):
    return _run(query, key, value, wq, bq, wk, bk, wv, bv, wo, bo, is_causal)[0]


def _run(query, key, value, wq, bq, wk, bk, wv, bv, wo, bo, is_causal, **run_kwargs):
    query = np.asarray(query, dtype=np.float32)
    key = np.asarray(key, dtype=np.float32)
    value = np.asarray(value, dtype=np.float32)
    wq, wk, wv, wo = (np.asarray(w, dtype=np.float32) for w in (wq, wk, wv, wo))
    bq, bk, bv, bo = (np.asarray(b, dtype=np.float32) for b in (bq, bk, bv, bo))
    causal = bool(int(np.asarray(is_causal)))

    if causal not in _BUILT:
        _BUILT[causal] = _build(causal)
    nc = _BUILT[causal]

    # triangular diagonal mask: mask[kp, qf] = 1 if qf >= kp
    qf = np.arange(P)[None, :]
    kp = np.arange(P)[:, None]
    masks_np = (qf >= kp).astype(np.float16)                  # [128, 128]

    in_maps = []
    for c in range(NCORES):
        b = c // 2
        g = c % 2
        hs = slice(g * DQ, (g + 1) * DQ)
        m = {
            "qT": _chunked(query[b].T.astype(np.float16), P),
            "kT": _chunked(key[b].T.astype(np.float16), P),
            "vT": _chunked(value[b].T.astype(np.float16), P),
            "wqT": _chunked(wq[hs, :].T.astype(np.float16), P),
            "wkT": _chunked(wk[hs, :].T.astype(np.float16), P),
            "wvT": _chunked(wv[hs, :].T.astype(np.float16), P),
            "woT": _chunked(wo[:, hs].T.astype(np.float16), P),
            "bq": np.ascontiguousarray(bq[hs].reshape(NDC, P).T.astype(np.float32)),
            "bk": np.ascontiguousarray(bk[hs].reshape(NDC, P).T.astype(np.float32)),
            "bv": bv[hs].reshape(1, DQ).astype(np.float16),
            "bo": (bo if g == 0 else np.zeros_like(bo)).reshape(1, E).astype(np.float16),
            "masks": masks_np,
        }
        in_maps.append(m)

    res = run_bass_kernel_spmd(nc, in_maps, core_ids=list(range(NCORES)), **run_kwargs)

    out = np.empty((B, L, E), dtype=np.float32)
    for b in range(B):
        out[b] = res.results[2 * b]["out"] + res.results[2 * b + 1]["out"]
    return out, res
